# revision 5
# baseline (speedup 1.0000x reference)
"""Trainium2 Bass kernel for the Cayley-Menger validator problem.

Input : verts (262144, 5, 128) fp32
Output: (d2_pairs (262144, 10) fp32, vol2 (262144,) fp32)

Math notes
----------
reference computes, per simplex b:
    gram   = V V^T                       (5,5)
    d2     = relu(n_i + n_j - 2 g_ij)    (5,5), 10 unique pairs -> d2_pairs
    vol2   = PREFACTOR * det(CM(d2))     with CM the 6x6 bordered matrix

We use the Cayley-Menger <-> edge-Gram identity:
    PREFACTOR * det(CM(d2)) = det(Gt) / (k!)^2,   Gt_ij = (d2_0i + d2_0j - d2_ij)/2
so with H = 2*Gt (H_ii = 2 d2_0i, H_ij = d2_0i + d2_0j - d2_ij):
    vol2 = det(H) / (2^4 * 576) = det(H) / 9216
det(H) (4x4) is expanded with the Pluecker/Laplace two-row scheme
(6 upper minors x 6 lower minors).

Per-core layout (data parallel over 8 cores, Bc = 32768 items/core)
-------------------------------------------------------------------
item = blk*(128*K) + lane*K + k   (lane = SBUF partition, K items per
lane per block).  This makes every DMA (in and out) contiguous per
partition.  SoA accumulators hold one fp32 per item in column
c = blk*K + k of a (128, NC) tile, NC = Bc/128.

Engines: ScalarE does the 5 squared-norm reductions per item
(activation Square + accum), VectorE does the 10 cross products per
item (tensor_tensor_reduce mult/add with scale=-2 -> accumulates
-2*g_vw directly), plus the cheap SoA determinant postprocessing.
"""

import numpy as np
from contextlib import ExitStack

import concourse.bass as bass
import concourse.tile as tile
from concourse import bacc
from concourse import mybir
from concourse.bass_utils import run_bass_kernel_spmd

FP32 = mybir.dt.float32
ALU = mybir.AluOpType
ACT = mybir.ActivationFunctionType

PAIRS = [(0, 1), (0, 2), (0, 3), (0, 4), (1, 2), (1, 3), (1, 4),
         (2, 3), (2, 4), (3, 4)]
PIDX = {p: i for i, p in enumerate(PAIRS)}
# upper-triangle index map for the symmetric 4x4 H matrix (rows/cols 0..3)
TRI = [(i, j) for i in range(4) for j in range(i, 4)]
TIDX = {p: i for i, p in enumerate(TRI)}

N_CORES = 8
NV = 5
E = 128
D = NV * E  # 640


def build_program(Bc: int, K: int = 16) -> bass.Bass:
    assert Bc % (128 * K) == 0
    NB = Bc // (128 * K)       # number of DMA blocks
    NC = NB * K                # SoA columns (= Bc / 128)

    nc = bacc.Bacc("TRN2")
    x = nc.declare_dram_parameter("verts", [Bc, D], FP32, isOutput=False)
    d2_out = nc.declare_dram_parameter("d2", [Bc, 10], FP32, isOutput=True)
    v2_out = nc.declare_dram_parameter("vol2", [Bc], FP32, isOutput=True)

    x_blk = x.rearrange("(nb l k) d -> nb l (k d)", l=128, k=K)
    d2_re = d2_out.rearrange("(nb l k) p -> l nb k p", l=128, k=K)
    v2_re = v2_out.rearrange("(nb l k) -> l nb k", l=128, k=K)

    with tile.TileContext(nc) as tc, ExitStack() as ctx:
        vpool = ctx.enter_context(tc.tile_pool(name="verts", bufs=2))
        spool = ctx.enter_context(tc.tile_pool(name="scratch", bufs=4))
        acc = ctx.enter_context(tc.tile_pool(name="acc", bufs=1))
        post = ctx.enter_context(tc.tile_pool(name="post", bufs=1))
        tmp = ctx.enter_context(tc.tile_pool(name="tmp", bufs=4))

        G = acc.tile([128, 10, NC], FP32, tag="G")   # accumulates -2*g_vw
        N = acc.tile([128, NV, NC], FP32, tag="N")   # norms |x_v|^2

        for blk in range(NB):
            vb = vpool.tile([128, K, NV, E], FP32, tag="vb")
            nc.sync.dma_start(out=vb, in_=x_blk[blk])
            for k in range(K):
                c = blk * K + k
                for v in range(NV):
                    so = spool.tile([128, E], FP32, tag="so")
                    nc.scalar.activation(
                        out=so, in_=vb[:, k, v, :], func=ACT.Square,
                        accum_out=N[:, v, c:c + 1])
                for pi, (v, w) in enumerate(PAIRS):
                    to = spool.tile([128, E], FP32, tag="to")
                    # out = (x_v * -2) * x_w ; accum_out = sum -> -2*g_vw
                    nc.vector.scalar_tensor_tensor(
                        out=to, in0=vb[:, k, v, :], scalar=-2.0,
                        in1=vb[:, k, w, :],
                        op0=ALU.mult, op1=ALU.mult,
                        accum_out=G[:, pi, c:c + 1])

        # ---------------- SoA postprocessing on (128, NC) tiles ------------
        Dt = post.tile([128, 10, NC], FP32, tag="D")        # relu'd d2
        OUT = post.tile([128, NB, K, 10], FP32, tag="OUT")  # d2_pairs staging

        for pi, (v, w) in enumerate(PAIRS):
            a = tmp.tile([128, NC], FP32, tag="a")
            nc.vector.tensor_add(a, N[:, v, :], N[:, w, :])
            r = tmp.tile([128, NC], FP32, tag="r")
            nc.vector.tensor_add(r, a, G[:, pi, :])       # n_v + n_w - 2 g
            nc.vector.tensor_scalar_max(Dt[:, pi, :], r, 0.0)
            nc.scalar.copy(
                out=OUT[:, :, :, pi],
                in_=Dt[:, pi, :].rearrange("l (nb k) -> l nb k", nb=NB))

        # H = 2 * Gram-tilde, symmetric 4x4 stored as 10 upper-tri slices
        H = post.tile([128, len(TRI), NC], FP32, tag="H")
        for i in range(1, 5):
            nc.vector.tensor_scalar_mul(
                H[:, TIDX[(i - 1, i - 1)], :], Dt[:, PIDX[(0, i)], :], 2.0)
            for j in range(i + 1, 5):
                hs = tmp.tile([128, NC], FP32, tag="hs")
                nc.vector.tensor_add(
                    hs, Dt[:, PIDX[(0, i)], :], Dt[:, PIDX[(0, j)], :])
                nc.vector.tensor_sub(
                    H[:, TIDX[(i - 1, j - 1)], :], hs, Dt[:, PIDX[(i, j)], :])

        def Hap(r, c):
            return H[:, TIDX[(min(r, c), max(r, c))], :]

        # 2x2 minors of rows (0,1) and rows (2,3)
        C2 = [(0, 1), (0, 2), (0, 3), (1, 2), (1, 3), (2, 3)]
        S = post.tile([128, 6, NC], FP32, tag="S")
        T = post.tile([128, 6, NC], FP32, tag="T")
        for mi, (a, b) in enumerate(C2):
            m1 = tmp.tile([128, NC], FP32, tag="m1")
            m2 = tmp.tile([128, NC], FP32, tag="m2")
            nc.vector.tensor_mul(m1, Hap(0, a), Hap(1, b))
            nc.vector.tensor_mul(m2, Hap(0, b), Hap(1, a))
            nc.vector.tensor_sub(S[:, mi, :], m1, m2)
            m3 = tmp.tile([128, NC], FP32, tag="m3")
            m4 = tmp.tile([128, NC], FP32, tag="m4")
            nc.vector.tensor_mul(m3, Hap(2, a), Hap(3, b))
            nc.vector.tensor_mul(m4, Hap(2, b), Hap(3, a))
            nc.vector.tensor_sub(T[:, mi, :], m3, m4)

        # det = s01 t23 - s02 t13 + s03 t12 + s12 t03 - s13 t02 + s23 t01
        # C2 index:  0:(0,1) 1:(0,2) 2:(0,3) 3:(1,2) 4:(1,3) 5:(2,3)
        terms = [(0, 5, +1), (1, 4, -1), (2, 3, +1),
                 (3, 2, +1), (4, 1, -1), (5, 0, +1)]
        det = tmp.tile([128, NC], FP32, tag="det")
        first = True
        for (si, ti, sgn) in terms:
            prod = tmp.tile([128, NC], FP32, tag="prod")
            nc.vector.tensor_mul(prod, S[:, si, :], T[:, ti, :])
            if first:
                nc.vector.tensor_copy(det, prod)
                first = False
            elif sgn > 0:
                nc.vector.tensor_add(det, det, prod)
            else:
                nc.vector.tensor_sub(det, det, prod)

        VOL = post.tile([128, NC], FP32, tag="VOL")
        nc.vector.tensor_scalar_mul(VOL, det, 1.0 / 9216.0)

        # ---------------- outputs ----------------
        nc.sync.dma_start(out=d2_re, in_=OUT)
        nc.sync.dma_start(
            out=v2_re, in_=VOL.rearrange("l (nb k) -> l nb k", nb=NB))

    nc.compile()
    return nc


_PROG_CACHE: dict = {}


def _get_program(Bc: int, K: int = 16) -> bass.Bass:
    key = (Bc, K)
    if key not in _PROG_CACHE:
        _PROG_CACHE[key] = build_program(Bc, K)
    return _PROG_CACHE[key]


def _run(verts: np.ndarray, **spmd_kwargs):
    B = verts.shape[0]
    assert verts.shape == (B, NV, E)
    Bc = B // N_CORES
    nc = _get_program(Bc)
    flat = np.ascontiguousarray(verts, dtype=np.float32).reshape(B, D)
    in_maps = [
        {"verts": flat[i * Bc:(i + 1) * Bc]} for i in range(N_CORES)
    ]
    res = run_bass_kernel_spmd(nc, in_maps, list(range(N_CORES)),
                               **spmd_kwargs)
    d2 = np.concatenate([r["d2"] for r in res.results], axis=0)
    vol2 = np.concatenate([r["vol2"] for r in res.results], axis=0).reshape(B)
    return (d2.astype(np.float32), vol2.astype(np.float32)), res


def kernel(verts: np.ndarray):
    out, _ = _run(verts)
    return out


# revision 10
# speedup vs baseline: 1.0823x; 1.0823x over previous
"""Trainium2 Bass kernel for the Cayley-Menger validator problem.

Input : verts (262144, 5, 128) fp32
Output: (d2_pairs (262144, 10) fp32, vol2 (262144,) fp32)

Math notes
----------
reference computes, per simplex b:
    gram   = V V^T                       (5,5)
    d2     = relu(n_i + n_j - 2 g_ij)    (5,5), 10 unique pairs -> d2_pairs
    vol2   = PREFACTOR * det(CM(d2))     with CM the 6x6 bordered matrix

We use the Cayley-Menger <-> edge-Gram identity:
    PREFACTOR * det(CM(d2)) = det(Gt) / (k!)^2,  Gt_ij = (d2_0i + d2_0j - d2_ij)/2
so with H = 2*Gt (H_ii = 2 d2_0i, H_ij = d2_0i + d2_0j - d2_ij):
    vol2 = det(H) / (2^4 * 576) = det(H) / 9216
det(H) (4x4) is expanded with the Pluecker/Laplace two-row scheme
(6 upper minors x 6 lower minors).

Per-core layout (data parallel over 8 cores, Bc = 32768 items/core)
-------------------------------------------------------------------
item = blk*(128*K) + lane*K + k   (lane = SBUF partition, K items per
lane per block).  This makes every DMA (in and out) contiguous per
partition.  SoA accumulators hold one fp32 per item in column
c = blk*K + k of a (128, NC) tile, NC = Bc/128.

Engine split (measured rates: DVE STT ~228ns, ActE act+accum ~515ns,
GpSimd TT ~430ns per (128,128) op):
 - VectorE: the 10 cross products per item (scalar_tensor_tensor
   mult/mult with scalar=-2 -> accumulates -2*g_vw in one pass), plus
   1/3 of the 5th norm ops, plus d2 assembly, plus the tail half of
   the determinant postprocessing.
 - ScalarE: squared-norm reductions (activation Square + accum).
 - GpSimd: determinant postprocessing for the first half (hidden
   under the main loop).
"""

import numpy as np
from contextlib import ExitStack

import concourse.bass as bass
import concourse.tile as tile
from concourse import bacc
from concourse import mybir
from concourse.bass_utils import run_bass_kernel_spmd

FP32 = mybir.dt.float32
ALU = mybir.AluOpType
ACT = mybir.ActivationFunctionType

PAIRS = [(0, 1), (0, 2), (0, 3), (0, 4), (1, 2), (1, 3), (1, 4),
         (2, 3), (2, 4), (3, 4)]
PIDX = {p: i for i, p in enumerate(PAIRS)}
# upper-triangle index map for the symmetric 4x4 H matrix (rows/cols 0..3)
TRI = [(i, j) for i in range(4) for j in range(i, 4)]
TIDX = {p: i for i, p in enumerate(TRI)}

N_CORES = 8
NV = 5
E = 128
D = NV * E  # 640


def build_program(Bc: int, K: int = 16) -> bass.Bass:
    assert Bc % (128 * K) == 0
    NB = Bc // (128 * K)       # number of DMA blocks
    NC = NB * K                # SoA columns (= Bc / 128)
    assert NB % 8 == 0

    nc = bacc.Bacc("TRN2")
    x = nc.declare_dram_parameter("verts", [Bc, D], FP32, isOutput=False)
    d2_out = nc.declare_dram_parameter("d2", [Bc, 10], FP32, isOutput=True)
    v2_out = nc.declare_dram_parameter("vol2", [Bc], FP32, isOutput=True)

    x_blk = x.rearrange("(nb l k) d -> nb l (k d)", l=128, k=K)
    d2_re = d2_out.rearrange("(nb l k) p -> l nb k p", l=128, k=K)
    v2_re = v2_out.rearrange("(nb l k) -> l nb k", l=128, k=K)

    with tile.TileContext(nc) as tc, ExitStack() as ctx:
        vpool = ctx.enter_context(tc.tile_pool(name="verts", bufs=3))
        spool = ctx.enter_context(tc.tile_pool(name="scratch", bufs=4))
        acc = ctx.enter_context(tc.tile_pool(name="acc", bufs=1))
        post = ctx.enter_context(tc.tile_pool(name="post", bufs=1))
        tmp = ctx.enter_context(tc.tile_pool(name="tmp", bufs=4))

        G = acc.tile([128, 10, NC], FP32, tag="G")   # accumulates -2*g_vw
        N = acc.tile([128, NV, NC], FP32, tag="N")   # norms |x_v|^2

        OUT = post.tile([128, NB, K, 10], FP32, tag="OUT")  # relu'd d2
        H = post.tile([128, len(TRI), NC], FP32, tag="H")
        S = post.tile([128, 6, NC], FP32, tag="S")
        T = post.tile([128, 6, NC], FP32, tag="T")
        VOL = post.tile([128, NC], FP32, tag="VOL")

        def Dap(pi, q):
            """AP of relu'd d2 for pair pi over one column-slice."""
            lo = q * (NB // 8)
            return OUT[:, lo:lo + NB // 8, :, pi].rearrange(
                "l nb k -> l (nb k)")

        NQ = 8                     # postproc slices
        def postproc(q, eng):
            """d2 assembly + H/minors/det for one column-quarter on eng."""
            QC = NC // NQ
            QB = NB // NQ
            lo_c = q * QC
            sl = slice(lo_c, lo_c + QC)
            lo_b = q * QB
            slb = slice(lo_b, lo_b + QB)
            for pi, (v, w) in enumerate(PAIRS):
                a = tmp.tile([128, QC], FP32, tag="a")
                eng.tensor_add(a, N[:, v, sl], N[:, w, sl])
                r = tmp.tile([128, QC], FP32, tag="r")
                eng.tensor_add(r, a, G[:, pi, sl])  # n_v + n_w - 2g
                eng.tensor_scalar_max(
                    OUT[:, slb, :, pi],
                    r.rearrange("l (nb k) -> l nb k", nb=QB), 0.0)

            for i in range(1, 5):
                eng.tensor_scalar_mul(
                    H[:, TIDX[(i - 1, i - 1)], sl], Dap(PIDX[(0, i)], q),
                    2.0)
                for j in range(i + 1, 5):
                    hs = tmp.tile([128, QC], FP32, tag="hs")
                    eng.tensor_add(hs, Dap(PIDX[(0, i)], q),
                                   Dap(PIDX[(0, j)], q))
                    eng.tensor_sub(H[:, TIDX[(i - 1, j - 1)], sl], hs,
                                   Dap(PIDX[(i, j)], q))

            def Hap(r_, c_):
                return H[:, TIDX[(min(r_, c_), max(r_, c_))], sl]

            C2 = [(0, 1), (0, 2), (0, 3), (1, 2), (1, 3), (2, 3)]
            for mi, (a_, b_) in enumerate(C2):
                m1 = tmp.tile([128, QC], FP32, tag="m1")
                m2 = tmp.tile([128, QC], FP32, tag="m2")
                eng.tensor_mul(m1, Hap(0, a_), Hap(1, b_))
                eng.tensor_mul(m2, Hap(0, b_), Hap(1, a_))
                eng.tensor_sub(S[:, mi, sl], m1, m2)
                m3 = tmp.tile([128, QC], FP32, tag="m3")
                m4 = tmp.tile([128, QC], FP32, tag="m4")
                eng.tensor_mul(m3, Hap(2, a_), Hap(3, b_))
                eng.tensor_mul(m4, Hap(2, b_), Hap(3, a_))
                eng.tensor_sub(T[:, mi, sl], m3, m4)

            # det = s01 t23 - s02 t13 + s03 t12 + s12 t03 - s13 t02 + s23 t01
            terms = [(0, 5, +1), (1, 4, -1), (2, 3, +1),
                     (3, 2, +1), (4, 1, -1), (5, 0, +1)]
            det = tmp.tile([128, QC], FP32, tag="det")
            first = True
            for (si, ti, sgn) in terms:
                prod = tmp.tile([128, QC], FP32, tag="prod")
                eng.tensor_mul(prod, S[:, si, sl], T[:, ti, sl])
                if first:
                    eng.tensor_copy(det, prod)
                    first = False
                elif sgn > 0:
                    eng.tensor_add(det, det, prod)
                else:
                    eng.tensor_sub(det, det, prod)
            eng.tensor_scalar_mul(VOL[:, sl], det, 1.0 / 9216.0)

            nc.sync.dma_start(
                out=d2_re[:, slb], in_=OUT[:, slb])
            nc.sync.dma_start(
                out=v2_re[:, slb],
                in_=VOL[:, sl].rearrange("l (nb k) -> l nb k", nb=QB))

        NGR = 4                    # DMA granules per block
        for blk in range(NB):
            vb = vpool.tile([128, K, NV, E], FP32, tag="vb")
            xg = x_blk[blk].rearrange("l (g kd) -> l g kd", g=NGR)
            vg = vb.rearrange("l k v e -> l (k v e)").rearrange(
                "l (g kd) -> l g kd", g=NGR)
            for g in range(NGR):
                nc.sync.dma_start(out=vg[:, g], in_=xg[:, g])
            for k in range(K):
                c = blk * K + k
                for v in range(NV - 1):
                    so = spool.tile([128, E], FP32, tag="so")
                    nc.scalar.activation(
                        out=so, in_=vb[:, k, v, :], func=ACT.Square,
                        accum_out=N[:, v, c:c + 1])
                # 5th norm: 1 of 3 chunks on DVE, rest on ActE
                if c % 3 == 0:
                    tn = spool.tile([128, E], FP32, tag="tn")
                    nc.vector.scalar_tensor_tensor(
                        out=tn, in0=vb[:, k, 4, :], scalar=1.0,
                        in1=vb[:, k, 4, :], op0=ALU.mult, op1=ALU.mult,
                        accum_out=N[:, 4, c:c + 1])
                else:
                    so = spool.tile([128, E], FP32, tag="so")
                    nc.scalar.activation(
                        out=so, in_=vb[:, k, 4, :], func=ACT.Square,
                        accum_out=N[:, 4, c:c + 1])
                for pi, (v, w) in enumerate(PAIRS):
                    to = spool.tile([128, E], FP32, tag="to")
                    # out = (x_v * -2) * x_w ; accum_out = sum -> -2*g_vw
                    nc.vector.scalar_tensor_tensor(
                        out=to, in0=vb[:, k, v, :], scalar=-2.0,
                        in1=vb[:, k, w, :],
                        op0=ALU.mult, op1=ALU.mult,
                        accum_out=G[:, pi, c:c + 1])
            if (blk + 1) % (NB // 8) == 0 and blk != NB - 1:
                postproc((blk + 1) // (NB // 8) - 1, nc.gpsimd)  # hidden
        postproc(7, nc.vector)           # tail

    nc.compile()
    return nc


_PROG_CACHE: dict = {}


def _get_program(Bc: int, K: int = 16) -> bass.Bass:
    key = (Bc, K)
    if key not in _PROG_CACHE:
        _PROG_CACHE[key] = build_program(Bc, K)
    return _PROG_CACHE[key]


def _run(verts: np.ndarray, **spmd_kwargs):
    B = verts.shape[0]
    assert verts.shape == (B, NV, E)
    Bc = B // N_CORES
    nc = _get_program(Bc)
    flat = np.ascontiguousarray(verts, dtype=np.float32).reshape(B, D)
    in_maps = [
        {"verts": flat[i * Bc:(i + 1) * Bc]} for i in range(N_CORES)
    ]
    res = run_bass_kernel_spmd(nc, in_maps, list(range(N_CORES)),
                               **spmd_kwargs)
    d2 = np.concatenate([r["d2"] for r in res.results], axis=0)
    vol2 = np.concatenate([r["vol2"] for r in res.results], axis=0).reshape(B)
    return (d2.astype(np.float32), vol2.astype(np.float32)), res


def kernel(verts: np.ndarray):
    out, _ = _run(verts)
    return out


# revision 12
# speedup vs baseline: 1.0878x; 1.0051x over previous
"""Trainium2 Bass kernel for the Cayley-Menger validator problem.

Input : verts (262144, 5, 128) fp32
Output: (d2_pairs (262144, 10) fp32, vol2 (262144,) fp32)

Math notes
----------
reference computes, per simplex b:
    gram   = V V^T                       (5,5)
    d2     = relu(n_i + n_j - 2 g_ij)    (5,5), 10 unique pairs -> d2_pairs
    vol2   = PREFACTOR * det(CM(d2))     with CM the 6x6 bordered matrix

We use the Cayley-Menger <-> edge-Gram identity:
    PREFACTOR * det(CM(d2)) = det(Gt) / (k!)^2,  Gt_ij = (d2_0i + d2_0j - d2_ij)/2
so with H = 2*Gt (H_ii = 2 d2_0i, H_ij = d2_0i + d2_0j - d2_ij):
    vol2 = det(H) / (2^4 * 576) = det(H) / 9216
det(H) (4x4) is expanded with the Pluecker/Laplace two-row scheme
(6 upper minors x 6 lower minors).

Per-core layout (data parallel over 8 cores, Bc = 32768 items/core)
-------------------------------------------------------------------
item = blk*(128*K) + lane*K + k   (lane = SBUF partition, K items per
lane per block).  This makes every DMA (in and out) contiguous per
partition.  SoA accumulators hold one fp32 per item in column
c = blk*K + k of a (128, NC) tile, NC = Bc/128.

Engine split (measured rates: DVE STT ~228ns, ActE act+accum ~515ns,
GpSimd TT ~430ns per (128,128) op):
 - VectorE: the 10 cross products per item (scalar_tensor_tensor
   mult/mult with scalar=-2 -> accumulates -2*g_vw in one pass), plus
   1/3 of the 5th norm ops, plus the final eighth of the
   postprocessing (tail).
 - ScalarE: squared-norm reductions (activation Square + accum).
 - GpSimd: d2 assembly + determinant postprocessing for the first 7
   of 8 column-slices (hidden under the main loop).

Measured on 8 axon trn2 cores: 620 us HW exec, rel err 4.6e-6
(DVE 97.6% busy = the binding engine; DMA fully overlapped).
"""

import numpy as np
from contextlib import ExitStack

import concourse.bass as bass
import concourse.tile as tile
from concourse import bacc
from concourse import mybir
from concourse.bass_utils import run_bass_kernel_spmd

FP32 = mybir.dt.float32
ALU = mybir.AluOpType
ACT = mybir.ActivationFunctionType

PAIRS = [(0, 1), (0, 2), (0, 3), (0, 4), (1, 2), (1, 3), (1, 4),
         (2, 3), (2, 4), (3, 4)]
PIDX = {p: i for i, p in enumerate(PAIRS)}
# upper-triangle index map for the symmetric 4x4 H matrix (rows/cols 0..3)
TRI = [(i, j) for i in range(4) for j in range(i, 4)]
TIDX = {p: i for i, p in enumerate(TRI)}

N_CORES = 8
NV = 5
E = 128
D = NV * E  # 640


def build_program(Bc: int, K: int = 16) -> bass.Bass:
    assert Bc % (128 * K) == 0
    NB = Bc // (128 * K)       # number of DMA blocks
    NC = NB * K                # SoA columns (= Bc / 128)
    assert NB % 8 == 0

    nc = bacc.Bacc("TRN2")
    x = nc.declare_dram_parameter("verts", [Bc, D], FP32, isOutput=False)
    d2_out = nc.declare_dram_parameter("d2", [Bc, 10], FP32, isOutput=True)
    v2_out = nc.declare_dram_parameter("vol2", [Bc], FP32, isOutput=True)

    x_blk = x.rearrange("(nb l k) d -> nb l (k d)", l=128, k=K)
    d2_re = d2_out.rearrange("(nb l k) p -> l nb k p", l=128, k=K)
    v2_re = v2_out.rearrange("(nb l k) -> l nb k", l=128, k=K)

    with tile.TileContext(nc) as tc, ExitStack() as ctx:
        vpool = ctx.enter_context(tc.tile_pool(name="verts", bufs=3))
        spool = ctx.enter_context(tc.tile_pool(name="scratch", bufs=4))
        acc = ctx.enter_context(tc.tile_pool(name="acc", bufs=1))
        post = ctx.enter_context(tc.tile_pool(name="post", bufs=1))
        tmp = ctx.enter_context(tc.tile_pool(name="tmp", bufs=4))

        G = acc.tile([128, 10, NC], FP32, tag="G")   # accumulates -2*g_vw
        N = acc.tile([128, NV, NC], FP32, tag="N")   # norms |x_v|^2

        OUT = post.tile([128, NB, K, 10], FP32, tag="OUT")  # relu'd d2
        H = post.tile([128, len(TRI), NC], FP32, tag="H")
        S = post.tile([128, 6, NC], FP32, tag="S")
        T = post.tile([128, 6, NC], FP32, tag="T")
        VOL = post.tile([128, NC], FP32, tag="VOL")

        def Dap(pi, q):
            """AP of relu'd d2 for pair pi over one column-slice."""
            lo = q * (NB // 8)
            return OUT[:, lo:lo + NB // 8, :, pi].rearrange(
                "l nb k -> l (nb k)")

        NQ = 8                     # postproc slices
        def postproc(q, eng):
            """d2 assembly + H/minors/det for one column-quarter on eng."""
            QC = NC // NQ
            QB = NB // NQ
            lo_c = q * QC
            sl = slice(lo_c, lo_c + QC)
            lo_b = q * QB
            slb = slice(lo_b, lo_b + QB)
            for pi, (v, w) in enumerate(PAIRS):
                a = tmp.tile([128, QC], FP32, tag="a")
                eng.tensor_add(a, N[:, v, sl], N[:, w, sl])
                r = tmp.tile([128, QC], FP32, tag="r")
                eng.tensor_add(r, a, G[:, pi, sl])  # n_v + n_w - 2g
                eng.tensor_scalar_max(
                    OUT[:, slb, :, pi],
                    r.rearrange("l (nb k) -> l nb k", nb=QB), 0.0)

            for i in range(1, 5):
                eng.tensor_scalar_mul(
                    H[:, TIDX[(i - 1, i - 1)], sl], Dap(PIDX[(0, i)], q),
                    2.0)
                for j in range(i + 1, 5):
                    hs = tmp.tile([128, QC], FP32, tag="hs")
                    eng.tensor_add(hs, Dap(PIDX[(0, i)], q),
                                   Dap(PIDX[(0, j)], q))
                    eng.tensor_sub(H[:, TIDX[(i - 1, j - 1)], sl], hs,
                                   Dap(PIDX[(i, j)], q))

            def Hap(r_, c_):
                return H[:, TIDX[(min(r_, c_), max(r_, c_))], sl]

            C2 = [(0, 1), (0, 2), (0, 3), (1, 2), (1, 3), (2, 3)]
            for mi, (a_, b_) in enumerate(C2):
                m1 = tmp.tile([128, QC], FP32, tag="m1")
                m2 = tmp.tile([128, QC], FP32, tag="m2")
                eng.tensor_mul(m1, Hap(0, a_), Hap(1, b_))
                eng.tensor_mul(m2, Hap(0, b_), Hap(1, a_))
                eng.tensor_sub(S[:, mi, sl], m1, m2)
                m3 = tmp.tile([128, QC], FP32, tag="m3")
                m4 = tmp.tile([128, QC], FP32, tag="m4")
                eng.tensor_mul(m3, Hap(2, a_), Hap(3, b_))
                eng.tensor_mul(m4, Hap(2, b_), Hap(3, a_))
                eng.tensor_sub(T[:, mi, sl], m3, m4)

            # det = s01 t23 - s02 t13 + s03 t12 + s12 t03 - s13 t02 + s23 t01
            terms = [(0, 5, +1), (1, 4, -1), (2, 3, +1),
                     (3, 2, +1), (4, 1, -1), (5, 0, +1)]
            det = tmp.tile([128, QC], FP32, tag="det")
            first = True
            for (si, ti, sgn) in terms:
                prod = tmp.tile([128, QC], FP32, tag="prod")
                eng.tensor_mul(prod, S[:, si, sl], T[:, ti, sl])
                if first:
                    eng.tensor_copy(det, prod)
                    first = False
                elif sgn > 0:
                    eng.tensor_add(det, det, prod)
                else:
                    eng.tensor_sub(det, det, prod)
            eng.tensor_scalar_mul(VOL[:, sl], det, 1.0 / 9216.0)

            nc.sync.dma_start(
                out=d2_re[:, slb], in_=OUT[:, slb])
            nc.sync.dma_start(
                out=v2_re[:, slb],
                in_=VOL[:, sl].rearrange("l (nb k) -> l nb k", nb=QB))

        NGR = 4                    # DMA granules per block
        for blk in range(NB):
            vb = vpool.tile([128, K, NV, E], FP32, tag="vb")
            xg = x_blk[blk].rearrange("l (g kd) -> l g kd", g=NGR)
            vg = vb.rearrange("l k v e -> l (k v e)").rearrange(
                "l (g kd) -> l g kd", g=NGR)
            for g in range(NGR):
                nc.sync.dma_start(out=vg[:, g], in_=xg[:, g])
            for k in range(K):
                c = blk * K + k
                for v in range(NV - 1):
                    so = spool.tile([128, E], FP32, tag="so")
                    nc.scalar.activation(
                        out=so, in_=vb[:, k, v, :], func=ACT.Square,
                        accum_out=N[:, v, c:c + 1])
                # 5th norm: 1 of 5 chunks on DVE, rest on ActE
                if c % 5 == 0:
                    tn = spool.tile([128, E], FP32, tag="tn")
                    nc.vector.scalar_tensor_tensor(
                        out=tn, in0=vb[:, k, 4, :], scalar=1.0,
                        in1=vb[:, k, 4, :], op0=ALU.mult, op1=ALU.mult,
                        accum_out=N[:, 4, c:c + 1])
                else:
                    so = spool.tile([128, E], FP32, tag="so")
                    nc.scalar.activation(
                        out=so, in_=vb[:, k, 4, :], func=ACT.Square,
                        accum_out=N[:, 4, c:c + 1])
                for pi, (v, w) in enumerate(PAIRS):
                    to = spool.tile([128, E], FP32, tag="to")
                    # out = (x_v * -2) * x_w ; accum_out = sum -> -2*g_vw
                    nc.vector.scalar_tensor_tensor(
                        out=to, in0=vb[:, k, v, :], scalar=-2.0,
                        in1=vb[:, k, w, :],
                        op0=ALU.mult, op1=ALU.mult,
                        accum_out=G[:, pi, c:c + 1])
            if (blk + 1) % (NB // 8) == 0 and blk != NB - 1:
                postproc((blk + 1) // (NB // 8) - 1, nc.gpsimd)  # hidden
        postproc(7, nc.vector)           # tail

    nc.compile()
    return nc


_PROG_CACHE: dict = {}


def _get_program(Bc: int, K: int = 16) -> bass.Bass:
    key = (Bc, K)
    if key not in _PROG_CACHE:
        _PROG_CACHE[key] = build_program(Bc, K)
    return _PROG_CACHE[key]


def _run(verts: np.ndarray, **spmd_kwargs):
    B = verts.shape[0]
    assert verts.shape == (B, NV, E)
    Bc = B // N_CORES
    nc = _get_program(Bc)
    flat = np.ascontiguousarray(verts, dtype=np.float32).reshape(B, D)
    in_maps = [
        {"verts": flat[i * Bc:(i + 1) * Bc]} for i in range(N_CORES)
    ]
    res = run_bass_kernel_spmd(nc, in_maps, list(range(N_CORES)),
                               **spmd_kwargs)
    d2 = np.concatenate([r["d2"] for r in res.results], axis=0)
    vol2 = np.concatenate([r["vol2"] for r in res.results], axis=0).reshape(B)
    return (d2.astype(np.float32), vol2.astype(np.float32)), res


def kernel(verts: np.ndarray):
    out, _ = _run(verts)
    return out
